# revision 14
# baseline (speedup 1.0000x reference)
"""BasicAttention Trainium2 kernel: fp8-DoubleRow Q/K/scores + pairwise
K/V dedup via AllGather + host-side pre-cast/pre-transpose of all inputs.

Reference (per batch b):
    q = x[b] @ Wq + bq; k = x[b] @ Wk + bk; v = x[b] @ Wv + bv
    s = q @ k.T / QD;  w = softmax(where(mask==0, -inf, s));  out = w @ v

Sharding: 8 cores = 4 batches x 2 query-halves. Each core projects K and V
only for its OWN 1024 rows and swaps halves with its partner through
pairwise AllGathers (DRAM bounce), overlapped with V-proj/Q-proj/scores.
The gather output is recomposed in RANK order (slot0 = rows 0:1024,
slot1 = rows 1024:2048 of the batch), so the key axis is in global order
on both cores, the host mask needs no rotation, and the program is fully
symmetric SPMD.

Precision: Q-proj, K-proj and scores run fp8e4 DoubleRow (2 k-tiles per
pass, ~1.44x PE). Wq/Wk are pre-scaled x32 on host so their +-1/32 entries
escape fp8 subnormals; the 32*32 factor is folded into the exp scale.
The V path (V-proj, P@V) stays bf16: the output is a near-uniform average
over ~1024 keys, so fp8 noise on V or P would land ~unattenuated (~3.6%)
on the output, over the 2e-2 budget.

v2 scheduling (from trace analysis of the 194.8us v1):
- The CC stream is strictly serial with ~6-10us ncfw start latency per op.
  v1: barrier 21-35, tiny sync AG 46-50, K AGs 51-75, V AG 77-110. The
  tiny sync AG is dropped: K AG1 itself absorbs bring-up, freeing ~14us
  of stream so the V AG finishes ~95 instead of 110.
- K is exchanged in two dt-half pieces staged DURING K-proj (after dt3 /
  dt7 evictions), so the AG triggers fire ~10us earlier.
- v1's 16us scores stall: the V-readback dma_start instructions sat in
  the Scalar/Sync engine FIFOs ahead of later exp ACTs, and blocked on
  the 33us V AG. v2 emits all V readbacks AFTER the scores loop, and
  K readbacks BEFORE Q-proj (their AGs are long done by the time the
  scalar queue reaches them).
- v1's scores phase was DVE-serialized (mask-mult 1.35us + den-add 1.2us
  per kt > 1.93us PE cadence). v2 moves the den accumulation to gpsimd,
  leaving DVE with the mask-mult only -> scores is PE-bound (~31us).
- V readback pieces are assigned to queues in PV consumption order:
  slot0 halves on sync (idle from ~60), slot1 on gpsimd/scalar (free
  right after scores).
"""

import sys

if "/opt/trn_rl_repo" not in sys.path:
    sys.path.insert(0, "/opt/trn_rl_repo")

import numpy as np

B, S_FULL, E_DIM, QD = 4, 2048, 1024, 1024
N_CORES = 8
P = 128
WSCALE = 32.0
# scores need exp(q.k/QD); q and k each carry x32 from weight pre-scaling
EXP_SCALE = 1.0 / (QD * WSCALE * WSCALE)


import contextlib


def _nullcm():
    return contextlib.nullcontext()


def _chunks(total, step):
    out = []
    c = 0
    while c < total:
        out.append((c, min(step, total - c)))
        c += step
    return out


def build_nc(S=2048, Sq=1024, E=1024, D=1024, use_cc=True):
    """Build + compile the per-core Bass program (symmetric SPMD)."""
    from contextlib import ExitStack

    import concourse.tile as tile
    from concourse import bacc, mybir

    bf16 = mybir.dt.bfloat16
    fp8 = mybir.dt.float8e4
    f32 = mybir.dt.float32
    AF = mybir.ActivationFunctionType
    ALU = mybir.AluOpType
    DR = mybir.MatmulPerfMode.DoubleRow

    NE = E // P   # e-chunks (8)
    ND = D // P   # d-tiles (8)
    NS = S // P   # key tiles (16: 8 per pair slot)
    NQ = Sq // P  # query tiles (8)
    NL = Sq // P  # local key tiles (8)
    NCH = 512     # psum bank chunk (fp32)
    NDH = ND // 2  # dt-half for the K exchange pieces (4)
    GROUPS = [[0, 1], [2, 3], [4, 5], [6, 7]]

    nc = bacc.Bacc("TRN2", target_bir_lowering=False, debug=False)

    # --- external inputs (host pre-laid-out) ---
    # all big inputs partition-major on host: [p, chunk, inner] -> 8-16KB
    # contiguous DRAM lines per partition (fast HWDGE)
    xt16_d = nc.dram_tensor("xt16", [P, NE * Sq], bf16, kind="ExternalInput").ap()
    xt8_d = nc.dram_tensor("xt8", [P, NE * Sq], fp8, kind="ExternalInput").ap()
    # mask in bf16: a bf16 x bf16 tensor_tensor hits the DVE 2x perf mode
    # (~0.69us per [P,1024] vs ~1.5us with an fp8 operand)
    maskt_d = nc.dram_tensor("maskt", [P, NS * Sq], bf16, kind="ExternalInput").ap()
    wq8_d = nc.dram_tensor("wq8", [P, NE * D], fp8, kind="ExternalInput").ap()
    wk8_d = nc.dram_tensor("wk8", [P, NE * D], fp8, kind="ExternalInput").ap()
    wv16_d = nc.dram_tensor("wv16", [P, NE * D], bf16, kind="ExternalInput").ap()
    # bqk32[p, 0:ND] = 32*bq[dt*128+p]; [p, ND:2ND] = 32*bk[...]
    bqk_d = nc.dram_tensor("bqk32", [P, 2 * ND], f32, kind="ExternalInput").ap()
    # bv replicated across partitions on host (DVE can't partition-broadcast)
    bv_d = nc.dram_tensor("bv16", [P, D], bf16, kind="ExternalInput").ap()
    out_d = nc.dram_tensor("out", [Sq, D], f32, kind="ExternalOutput").ap()

    # --- collective bounce buffers (internal DRAM) ---
    # sync AG: input is never written (content is irrelevant); its only job
    # is to fire the first collective trigger at t~6us so the device-wide
    # CC bring-up barrier (~17us, gated on ALL ranks' first triggers)
    # completes before the K AGs need the stream.
    ccS_in = nc.dram_tensor("ccS_in", [1, 32], f32, kind="Internal").ap()
    ccS_out = nc.dram_tensor("ccS_out", [2, 32], f32, kind="Internal").ap()
    # single K exchange buffer: one AG op minimizes serial CC-stream time
    ccK_in = nc.dram_tensor("ccK_in", [P, ND * Sq], fp8, kind="Internal").ap()
    ccK_out = nc.dram_tensor("ccK_out", [2, P, ND * Sq], fp8, kind="Internal").ap()
    ccV_in = nc.dram_tensor("ccV_in", [P, NL * D], bf16, kind="Internal").ap()
    ccV_out = nc.dram_tensor("ccV_out", [2, P, NL * D], bf16, kind="Internal").ap()

    with ExitStack() as ctx:
        tc = ctx.enter_context(tile.TileContext(nc))

        const = ctx.enter_context(tc.tile_pool(name="const", bufs=1))
        xt16_pool = ctx.enter_context(tc.tile_pool(name="xt16", bufs=1))
        xt8_pool = ctx.enter_context(tc.tile_pool(name="xt8", bufs=1))
        w8_pool = ctx.enter_context(tc.tile_pool(name="w8", bufs=1))
        wv_pool = ctx.enter_context(tc.tile_pool(name="wv", bufs=1))
        qt_pool = ctx.enter_context(tc.tile_pool(name="qt", bufs=1))
        kt_pool = ctx.enter_context(tc.tile_pool(name="kt", bufs=1))
        v_pool = ctx.enter_context(tc.tile_pool(name="v", bufs=1))
        pst_pool = ctx.enter_context(tc.tile_pool(name="pst", bufs=1))
        maskt_pool = ctx.enter_context(tc.tile_pool(name="maskt", bufs=1))
        evict = ctx.enter_context(tc.tile_pool(name="evict", bufs=2))
        o_pool = ctx.enter_context(tc.tile_pool(name="o", bufs=2))
        den_pool = ctx.enter_context(tc.tile_pool(name="den", bufs=2))

        mm_psum = ctx.enter_context(tc.tile_pool(name="mm_psum", bufs=3, space="PSUM"))
        den_psum = ctx.enter_context(tc.tile_pool(name="den_psum", bufs=2, space="PSUM"))

        # ---- t~6us: sync AG fires the CC bring-up barrier ASAP (input is
        #      garbage DRAM, output unused; the trigger is all that matters)
        if use_cc:
            with nc.named_scope("sync_ag"):
                nc.gpsimd.collective_compute(
                    "AllGather", ALU.bypass, replica_groups=GROUPS,
                    ins=[ccS_in[:, :].opt()], outs=[ccS_out[:, :].opt()],
                )

        # constants
        ones_col = const.tile([P, 1], f32)
        nc.vector.memset(ones_col[:, 0:1], 1.0)
        bqk_t = const.tile([P, 2 * ND], f32, name="bqk")
        nc.gpsimd.dma_start(out=bqk_t[:, :], in_=bqk_d[:, :])
        bv_t = const.tile([P, D], bf16)
        nc.gpsimd.dma_start(out=bv_t[:, :], in_=bv_d[:, :])

        # ---- PE warm-up: ~20 junk matmuls on a memset tile during the
        #      input-load window so HAM un-throttles (K=8/8) before K-proj.
        warm = const.tile([P, 512], bf16, name="warm")
        nc.vector.memset(warm[:, :], 0.0)
        wps = mm_psum.tile([P, 512], f32, tag="mm")
        for _ in range(20):
            nc.tensor.matmul(
                wps[:, 0:512], warm[:, 0:128], warm[:, 0:512],
                start=True, stop=True,
            )

        # persistent tensors
        xT16 = xt16_pool.tile([P, NE, Sq], bf16)   # x^T bf16 (V path)
        xT8 = xt8_pool.tile([P, NE, Sq], fp8)      # x^T fp8 (Q/K path)
        Wq8 = w8_pool.tile([P, NE, D], fp8)
        Wk8 = w8_pool.tile([P, NE, D], fp8)
        Wv = wv_pool.tile([P, NE, D], bf16)
        QT8 = qt_pool.tile([P, ND, Sq], fp8)       # Q^T fp8, x32-scaled
        KT8 = kt_pool.tile([P, ND, S], fp8)        # K^T fp8, x32-scaled
        V = v_pool.tile([P, NS, D], bf16)
        PsT = pst_pool.tile([P, NS, Sq], bf16)
        maskT = maskt_pool.tile([P, NS, Sq], bf16)
        # two denominator accumulators so the serial add chain splits
        # across DVE (even kt) and gpsimd (odd kt)
        den_a = den_pool.tile([P, Sq], f32, name="den_a")
        den_b = den_pool.tile([P, Sq], f32, name="den_b")

        # ---- input loads (queue placement = need time) ----
        with nc.named_scope("loads"):
            # K-proj gate = max(xT8, Wk8): split Wk8 across both HWDGE
            # queues so the gate lands ~2us earlier
            nc.sync.dma_start(out=xT8[:, :, :], in_=xt8_d[:, :])
            nc.scalar.dma_start(out=Wk8[:, 0:4, :], in_=wk8_d[:, 0 : 4 * D])
            nc.sync.dma_start(out=Wk8[:, 4:8, :], in_=wk8_d[:, 4 * D : 8 * D])
            nc.scalar.dma_start(out=Wq8[:, :, :], in_=wq8_d[:, :])
            # gpsimd SWDGE: V-path + mask loads
            nc.gpsimd.dma_start(out=xT16[:, :, :], in_=xt16_d[:, :])
            nc.gpsimd.dma_start(out=Wv[:, :, :], in_=wv16_d[:, :])
            nc.gpsimd.dma_start(out=maskT[:, :, :], in_=maskt_d[:, :])

        # ---- K-proj (fp8 DoubleRow): KT8[p,dt,s] = 32*(K[s, dt*128+p]+bk),
        #      local rows only, evicted straight to fp8. Each dt-half is
        #      staged for the AG as soon as its last eviction lands. ----
        def proj_fp8(dst, W8, bias_col0, span, scope, piece_hook=None):
            with nc.named_scope(scope):
                for dt in range(ND):
                    ps = mm_psum.tile([P, span], f32, tag="mm")
                    for ep in range(0, NE, 2):
                        for c0, cw in _chunks(span, NCH):
                            nc.tensor.matmul(
                                ps[:, c0 : c0 + cw],
                                W8[:, ep : ep + 2, dt * P : (dt + 1) * P],
                                xT8[:, ep : ep + 2, c0 : c0 + cw],
                                start=(ep == 0),
                                stop=(ep == NE - 2),
                                perf_mode=DR,
                            )
                    nc.scalar.activation(
                        dst[:, dt, 0:span],
                        ps[:, 0:span],
                        AF.Identity,
                        bias=bqk_t[:, bias_col0 + dt : bias_col0 + dt + 1],
                    )
                    if piece_hook is not None:
                        piece_hook(dt)

        proj_fp8(KT8, Wk8, ND, Sq, "KT")
        if use_cc:
            # stage the whole local K in two parallel half-DMAs, one AG op
            with nc.named_scope("kx"):
                nc.sync.dma_start(
                    out=ccK_in[:, : NDH * Sq].rearrange("p (dt s) -> p dt s", dt=NDH),
                    in_=KT8[:, 0:NDH, 0:Sq],
                )
                nc.scalar.dma_start(
                    out=ccK_in[:, NDH * Sq :].rearrange("p (dt s) -> p dt s", dt=NDH),
                    in_=KT8[:, NDH:ND, 0:Sq],
                )
                nc.gpsimd.collective_compute(
                    "AllGather",
                    ALU.bypass,
                    replica_groups=GROUPS,
                    ins=[ccK_in[:, :].opt()],
                    outs=[ccK_out[:, :, :].opt()],
                )

        # ---- V-proj (bf16): V[p, st, d], local st=0..7; stage each tile as
        #      soon as it's evicted, single AG for all 8 ----
        with nc.named_scope("V"):
            for st in range(NL):
                ps = mm_psum.tile([P, D], f32, tag="mm")
                for e in range(NE):
                    for c0, cw in _chunks(D, NCH):
                        nc.tensor.matmul(
                            ps[:, c0 : c0 + cw],
                            xT16[:, e, st * P : (st + 1) * P],
                            Wv[:, e, c0 : c0 + cw],
                            start=(e == 0),
                            stop=(e == NE - 1),
                        )
                nc.vector.tensor_tensor(
                    V[:, st, :], ps[:, 0:D], bv_t[:, :], op=ALU.add
                )
                if use_cc:
                    nc.gpsimd.dma_start(
                        out=ccV_in[:, st * D : (st + 1) * D], in_=V[:, st, :]
                    )
        if use_cc:
            with nc.named_scope("vx"):
                nc.gpsimd.collective_compute(
                    "AllGather", ALU.bypass, replica_groups=GROUPS,
                    ins=[ccV_in[:, :].opt()], outs=[ccV_out[:, :, :].opt()],
                )

        # ---- K gather readbacks: emitted BEFORE Q-proj so the scalar
        #      queue reaches them long after the AG finished (no FIFO
        #      block). Piece order = scores consumption order: slot0
        #      (kt 0..7) halves across both queues first, then slot1. ----
        with nc.named_scope("kin") if use_cc else _nullcm():
            for slot in range(2 if use_cc else 0):
                for j in range(2):
                    eng = nc.scalar if j == 0 else nc.sync
                    eng.dma_start(
                        out=KT8[
                            :, j * NDH : (j + 1) * NDH,
                            slot * Sq : (slot + 1) * Sq,
                        ],
                        in_=ccK_out[slot][
                            :, j * NDH * Sq : (j + 1) * NDH * Sq
                        ].rearrange("p (dt s) -> p dt s", dt=NDH),
                    )

        # ---- Q-proj (fp8 DoubleRow) while the K gather flies ----
        proj_fp8(QT8, Wq8, 0, Sq, "QT")

        # ---- scores (fp8 DoubleRow, transposed) + exp + mask; den
        #      accumulation on gpsimd so DVE only does the mask-mult ----
        with nc.named_scope("scores"):
            for kt in range(NS):
                ps = mm_psum.tile([P, Sq], f32, tag="mm")
                for dp in range(0, ND, 2):
                    for c0, cw in _chunks(Sq, NCH):
                        nc.tensor.matmul(
                            ps[:, c0 : c0 + cw],
                            KT8[:, dp : dp + 2, kt * P : (kt + 1) * P],
                            QT8[:, dp : dp + 2, c0 : c0 + cw],
                            start=(dp == 0),
                            stop=(dp == ND - 2),
                            perf_mode=DR,
                        )
                ex = evict.tile([P, Sq], bf16, tag="exp")
                nc.scalar.activation(ex[:, :], ps[:, 0:Sq], AF.Exp, scale=EXP_SCALE)
                nc.vector.tensor_tensor(
                    PsT[:, kt, :], ex[:, :], maskT[:, kt, :], op=ALU.mult
                )
                # gpsimd gets the EVEN kts (its slow initial cast lands
                # early; its last add is kt14, off the critical tail);
                # DVE's odd chain ends at kt15 just ~0.6us after mult15
                eng, den = (nc.gpsimd, den_a) if kt % 2 == 0 else (nc.vector, den_b)
                if kt < 2:
                    eng.tensor_copy(den[:, :], PsT[:, kt, :])
                else:
                    eng.tensor_tensor(
                        den[:, :], den[:, :], PsT[:, kt, :], op=ALU.add
                    )

        # ---- denominators (no V dependency) ----
        rdens = []
        with nc.named_scope("den"):
            for qt in range(NQ):
                dps = den_psum.tile([P, 1], f32, tag="den")
                nc.tensor.matmul(
                    dps[:, 0:1],
                    den_a[:, qt * P : (qt + 1) * P],
                    ones_col[:, 0:1],
                    start=True,
                    stop=False,
                )
                nc.tensor.matmul(
                    dps[:, 0:1],
                    den_b[:, qt * P : (qt + 1) * P],
                    ones_col[:, 0:1],
                    start=False,
                    stop=True,
                )
                rden = den_pool.tile([P, 1], f32, tag=f"rden{qt}", bufs=1)
                nc.vector.reciprocal(rden[:, 0:1], dps[:, 0:1])
                rdens.append(rden)

        # ---- V gather readbacks: emitted AFTER scores so their engine-
        #      queue slots can't block exp/mult issue. Queue choice =
        #      PV consumption order: sync is idle from ~60 so it carries
        #      slot0 (kt 0..7); gpsimd/scalar free right after scores
        #      carry slot1. ----
        NH = NL // 2
        with nc.named_scope("vin") if use_cc else _nullcm():
            engs = [nc.sync, nc.sync, nc.gpsimd, nc.scalar]
            for slot in range(2 if use_cc else 0):
                for hh in range(2):
                    engs[slot * 2 + hh].dma_start(
                        out=V[:, slot * NL + hh * NH : slot * NL + (hh + 1) * NH, :],
                        in_=ccV_out[slot, :, hh * NH * D : (hh + 1) * NH * D].rearrange(
                            "p (st d) -> p st d", st=NH
                        ),
                    )

        # ---- P@V per query tile (bf16) ----
        with nc.named_scope("pv"):
            for qt in range(NQ):
                ops = mm_psum.tile([P, D], f32, tag="mm")
                for kt in range(NS):
                    for c0, cw in _chunks(D, NCH):
                        nc.tensor.matmul(
                            ops[:, c0 : c0 + cw],
                            PsT[:, kt, qt * P : (qt + 1) * P],
                            V[:, kt, c0 : c0 + cw],
                            start=(kt == 0),
                            stop=(kt == NS - 1),
                        )
                ot = o_pool.tile([P, D], f32, tag="o")
                nc.scalar.activation(
                    ot[:, :], ops[:, 0:D], AF.Copy, scale=rdens[qt][:, 0:1]
                )
                if qt < NQ - 1:
                    eng = nc.sync if qt % 2 == 0 else nc.scalar
                    eng.dma_start(out=out_d[qt * P : (qt + 1) * P, :], in_=ot[:, :])
                else:
                    nc.sync.dma_start(
                        out=out_d[qt * P : qt * P + 64, :], in_=ot[0:64, :]
                    )
                    nc.scalar.dma_start(
                        out=out_d[qt * P + 64 : (qt + 1) * P, :], in_=ot[64:P, :]
                    )

    nc.compile()
    return nc


_NC_CACHE = {}


def _get_nc(key=(2048, 1024, 1024, 1024)):
    if key not in _NC_CACHE:
        _NC_CACHE[key] = build_nc(*key)
    return _NC_CACHE[key]


def shard_inputs(x, mask, Wq, bq, Wk, bk, Wv, bv):
    """Host-side prep: pre-transpose/pre-cast per-core inputs.

    The key axis on every core is the GLOBAL batch order (the AllGather
    recomposes K/V in rank order), so the mask is never rotated; each core
    takes its own query rows only.
    """
    import ml_dtypes

    fp8 = ml_dtypes.float8_e4m3
    bf16 = ml_dtypes.bfloat16
    Sq = x.shape[1] // 2
    ND = QD // P

    def pmajor(a):
        # [chunks*128, inner] -> [128, chunks*inner] partition-major
        n, inner = a.shape[0] // P, a.shape[1]
        return np.ascontiguousarray(
            a.reshape(n, P, inner).transpose(1, 0, 2).reshape(P, n * inner)
        )

    w8 = {
        "wq8": pmajor((Wq * WSCALE).astype(fp8)),
        "wk8": pmajor((Wk * WSCALE).astype(fp8)),
        "wv16": pmajor(Wv.astype(bf16)),
    }
    bqk32 = np.ascontiguousarray(
        np.concatenate(
            [(bq * WSCALE).reshape(ND, P).T, (bk * WSCALE).reshape(ND, P).T],
            axis=1,
        ).astype(np.float32)
    )
    bv16 = np.ascontiguousarray(
        np.broadcast_to(bv.reshape(1, -1), (P, bv.size)).astype(bf16)
    )

    in_maps = []
    for c in range(N_CORES):
        b, h = c // 2, c % 2
        xt = x[b, h * Sq : (h + 1) * Sq, :].T  # [E, Sq]
        maskt = mask[b, h * Sq : (h + 1) * Sq, :].T.astype(bf16)  # [S, Sq]
        in_maps.append(
            {
                "xt16": pmajor(xt.astype(bf16)),
                "xt8": pmajor(xt.astype(fp8)),
                "maskt": pmajor(maskt),
                "bqk32": bqk32,
                "bv16": bv16,
                **w8,
            }
        )
    return in_maps


def kernel(**inputs):
    """Full-problem entry point: full unsharded inputs -> full output."""
    from concourse.bass_utils import run_bass_kernel_spmd

    x = np.asarray(inputs["x"], dtype=np.float32)
    mask = np.asarray(inputs["mask"], dtype=np.int32)
    args = [
        np.asarray(inputs[k], dtype=np.float32)
        for k in ("Wq", "bq", "Wk", "bk", "Wv", "bv")
    ]

    nc = _get_nc()
    in_maps = shard_inputs(x, mask, *args)
    res = run_bass_kernel_spmd(nc, in_maps, core_ids=list(range(N_CORES)))

    Sq = S_FULL // 2
    out = np.empty((B, S_FULL, QD), dtype=np.float32)
    for c, r in enumerate(res.results):
        b, h = c // 2, c % 2
        out[b, h * Sq : (h + 1) * Sq, :] = r["out"]
    return out


# revision 34
# speedup vs baseline: 1.3919x; 1.3919x over previous
"""BasicAttention Trainium2 kernel: fp8-DoubleRow Q/K/scores + pairwise
K/V dedup via AllGather + host-side pre-cast/pre-transpose of all inputs.

Reference (per batch b):
    q = x[b] @ Wq + bq; k = x[b] @ Wk + bk; v = x[b] @ Wv + bv
    s = q @ k.T / QD;  w = softmax(where(mask==0, -inf, s));  out = w @ v

Sharding: 8 cores = 4 batches x 2 query-halves. Each core projects K and V
only for its OWN 1024 rows and swaps halves with its partner through
pairwise AllGathers (DRAM bounce), overlapped with V-proj/Q-proj/scores.
The gather output is recomposed in RANK order (slot0 = rows 0:1024,
slot1 = rows 1024:2048 of the batch), so the key axis is in global order
on both cores, the host mask needs no rotation, and the program is fully
symmetric SPMD.

Precision: Q-proj, K-proj and scores run fp8e4 DoubleRow (2 k-tiles per
pass, ~1.44x PE). Wq/Wk are pre-scaled x32 on host so their +-1/32 entries
escape fp8 subnormals; the 32*32 factor is folded into the exp scale.
The V path (V-proj, P@V) stays bf16: the output is a near-uniform average
over ~1024 keys, so fp8 noise on V or P would land ~unattenuated (~3.6%)
on the output, over the 2e-2 budget.

v7 scheduling (from ntff trace analysis; baseline v1 was ~195-200us,
v7 measures ~196.5us min / beats v1 by ~4us min-vs-min on same-day
conditions with structurally larger margins on good-barrier runs):
- CC stream model (measured): the bring-up BARRIER lasts 13-27us and its
  end gates on ALL ranks' first collective trigger; the first data op
  then pays ~11us ncfw latency; later ops chain back-to-back (+2us)
  ONLY if their trigger fired before the stream went idle. An UNSTAGED
  tiny sync AG (garbage input, output unused) is the very first gpsimd
  instruction, firing every rank's trigger at t~7us.
- One K AG (1MB) then one V AG (2MB), back-to-back on the stream. The V
  staging runs as 2 big HWDGE chunks on the SCALAR queue mid-V-proj so
  the V trigger fires (~65) well before the K AG completes (~70-79).
- Engine FIFOs are strict at runtime and Tile may REORDER same-queue
  DMAs, so any collective-gated readback must sit on a queue with no
  compute-path work due before the collective ends: K readbacks split
  sync+gpsimd, V readbacks split sync/gpsimd/scalar in PV consumption
  order, all emitted after the scores loop.
- Scores is PE-bound (~31us): mask in bf16 (bf16 x bf16 tensor_tensor
  hits the DVE 2x mode, 0.69us vs 1.5us with fp8) and the serial
  denominator accumulation splits into two chains (gpsimd: even kt,
  DVE: odd kt) merged by two accumulating 1-column matmuls per qt.
- ~20 junk warm-up matmuls during the input-load window flip HAM to
  K=8/8 before K-proj.
- Run-to-run variance on this part is +-8% (P0 downclock + barrier
  jitter); judge changes by min-of-3 (test.py REPS=3).
"""

import sys

if "/opt/trn_rl_repo" not in sys.path:
    sys.path.insert(0, "/opt/trn_rl_repo")

import numpy as np

B, S_FULL, E_DIM, QD = 4, 2048, 1024, 1024
N_CORES = 8
P = 128
WSCALE = 32.0
# scores need exp(q.k/QD); q and k each carry x32 from weight pre-scaling
EXP_SCALE = 1.0 / (QD * WSCALE * WSCALE)


import contextlib


def _nullcm():
    return contextlib.nullcontext()


def _chunks(total, step):
    out = []
    c = 0
    while c < total:
        out.append((c, min(step, total - c)))
        c += step
    return out


def build_nc(S=2048, Sq=1024, E=1024, D=1024, use_cc=True):
    """Build + compile the per-core Bass program (symmetric SPMD)."""
    from contextlib import ExitStack

    import concourse.tile as tile
    from concourse import bacc, mybir

    bf16 = mybir.dt.bfloat16
    fp8 = mybir.dt.float8e4
    f32 = mybir.dt.float32
    AF = mybir.ActivationFunctionType
    ALU = mybir.AluOpType
    DR = mybir.MatmulPerfMode.DoubleRow

    NE = E // P   # e-chunks (8)
    ND = D // P   # d-tiles (8)
    NS = S // P   # key tiles (16: 8 per pair slot)
    NQ = Sq // P  # query tiles (8)
    NL = Sq // P  # local key tiles (8)
    NCH = 512     # psum bank chunk (fp32)
    NDH = ND // 2  # dt-half for the K exchange pieces (4)
    GROUPS = [[0, 1], [2, 3], [4, 5], [6, 7]]

    nc = bacc.Bacc("TRN2", target_bir_lowering=False, debug=False)

    # --- external inputs (host pre-laid-out) ---
    # all big inputs partition-major on host: [p, chunk, inner] -> 8-16KB
    # contiguous DRAM lines per partition (fast HWDGE)
    xt16_d = nc.dram_tensor("xt16", [P, NE * Sq], bf16, kind="ExternalInput").ap()
    xt8_d = nc.dram_tensor("xt8", [P, NE * Sq], fp8, kind="ExternalInput").ap()
    # mask in bf16: a bf16 x bf16 tensor_tensor hits the DVE 2x perf mode
    # (~0.69us per [P,1024] vs ~1.5us with an fp8 operand)
    maskt_d = nc.dram_tensor("maskt", [P, NS * Sq], bf16, kind="ExternalInput").ap()
    wq8_d = nc.dram_tensor("wq8", [P, NE * D], fp8, kind="ExternalInput").ap()
    wk8_d = nc.dram_tensor("wk8", [P, NE * D], fp8, kind="ExternalInput").ap()
    wv16_d = nc.dram_tensor("wv16", [P, NE * D], bf16, kind="ExternalInput").ap()
    # bqk32[p, 0:ND] = 32*bq[dt*128+p]; [p, ND:2ND] = 32*bk[...]
    bqk_d = nc.dram_tensor("bqk32", [P, 2 * ND], f32, kind="ExternalInput").ap()
    # bv replicated across partitions on host (DVE can't partition-broadcast)
    bv_d = nc.dram_tensor("bv16", [P, D], bf16, kind="ExternalInput").ap()
    # per-core partner-slot row offsets for the indirect gather readbacks:
    # roff[p] = (1-h)*128 + p  (h = this core's pair half)
    roff_d = nc.dram_tensor("roff", [P, 1], mybir.dt.int32, kind="ExternalInput").ap()
    out_d = nc.dram_tensor("out", [Sq, D], f32, kind="ExternalOutput").ap()

    # --- collective bounce buffers (internal DRAM) ---
    # sync AG: input is never written (content is irrelevant); its only job
    # is to fire the first collective trigger at t~6us so the device-wide
    # CC bring-up barrier (~17us, gated on ALL ranks' first triggers)
    # completes before the K AGs need the stream.
    ccS_in = nc.dram_tensor("ccS_in", [1, 32], f32, kind="Internal").ap()
    ccS_out = nc.dram_tensor("ccS_out", [2, 32], f32, kind="Internal").ap()
    # single K exchange buffer: one AG op minimizes serial CC-stream time
    ccK_in = nc.dram_tensor("ccK_in", [P, ND * Sq], fp8, kind="Internal").ap()
    ccK_out = nc.dram_tensor("ccK_out", [2, P, ND * Sq], fp8, kind="Internal").ap()
    ccV_in = nc.dram_tensor("ccV_in", [P, NL * D], bf16, kind="Internal").ap()
    ccV_out = nc.dram_tensor("ccV_out", [2, P, NL * D], bf16, kind="Internal").ap()

    with ExitStack() as ctx:
        tc = ctx.enter_context(tile.TileContext(nc))

        const = ctx.enter_context(tc.tile_pool(name="const", bufs=1))
        xt16_pool = ctx.enter_context(tc.tile_pool(name="xt16", bufs=1))
        xt8_pool = ctx.enter_context(tc.tile_pool(name="xt8", bufs=1))
        w8_pool = ctx.enter_context(tc.tile_pool(name="w8", bufs=1))
        wv_pool = ctx.enter_context(tc.tile_pool(name="wv", bufs=1))
        qt_pool = ctx.enter_context(tc.tile_pool(name="qt", bufs=1))
        kt_pool = ctx.enter_context(tc.tile_pool(name="kt", bufs=1))
        v_pool = ctx.enter_context(tc.tile_pool(name="v", bufs=1))
        pst_pool = ctx.enter_context(tc.tile_pool(name="pst", bufs=1))
        maskt_pool = ctx.enter_context(tc.tile_pool(name="maskt", bufs=1))
        evict = ctx.enter_context(tc.tile_pool(name="evict", bufs=2))
        o_pool = ctx.enter_context(tc.tile_pool(name="o", bufs=2))
        den_pool = ctx.enter_context(tc.tile_pool(name="den", bufs=2))

        mm_psum = ctx.enter_context(tc.tile_pool(name="mm_psum", bufs=3, space="PSUM"))
        den_psum = ctx.enter_context(tc.tile_pool(name="den_psum", bufs=2, space="PSUM"))

        # ---- t~6us: sync AG fires the CC bring-up barrier ASAP (input is
        #      garbage DRAM, output unused; the trigger is all that matters)
        if use_cc:
            with nc.named_scope("sync_ag"):
                nc.gpsimd.collective_compute(
                    "AllGather", ALU.bypass, replica_groups=GROUPS,
                    ins=[ccS_in[:, :].opt()], outs=[ccS_out[:, :].opt()],
                )

        # constants
        ones_col = const.tile([P, 1], f32)
        nc.vector.memset(ones_col[:, 0:1], 1.0)
        bqk_t = const.tile([P, 2 * ND], f32, name="bqk")
        nc.gpsimd.dma_start(out=bqk_t[:, :], in_=bqk_d[:, :])
        bv_t = const.tile([P, D], bf16)
        nc.gpsimd.dma_start(out=bv_t[:, :], in_=bv_d[:, :])
        roff_t = const.tile([P, 1], mybir.dt.int32, name="roff")
        nc.gpsimd.dma_start(out=roff_t[:, :], in_=roff_d[:, :])

        # ---- PE warm-up: ~20 junk matmuls on a memset tile during the
        #      input-load window so HAM un-throttles (K=8/8) before K-proj.
        warm = const.tile([P, 512], bf16, name="warm")
        nc.vector.memset(warm[:, :], 0.0)
        wps = mm_psum.tile([P, 512], f32, tag="mm")
        for _ in range(20):
            nc.tensor.matmul(
                wps[:, 0:512], warm[:, 0:128], warm[:, 0:512],
                start=True, stop=True,
            )

        # persistent tensors
        xT16 = xt16_pool.tile([P, NE, Sq], bf16)   # x^T bf16 (V path)
        xT8 = xt8_pool.tile([P, NE, Sq], fp8)      # x^T fp8 (Q/K path)
        Wq8 = w8_pool.tile([P, NE, D], fp8)
        Wk8 = w8_pool.tile([P, NE, D], fp8)
        Wv = wv_pool.tile([P, NE, D], bf16)
        QT8 = qt_pool.tile([P, ND, Sq], fp8)       # Q^T fp8, x32-scaled
        # own K/V written by the projections; partner K/V land in separate
        # CONTIGUOUS 2D tiles via per-partition indirect gathers
        KT8 = kt_pool.tile([P, ND, Sq], fp8)       # own K^T fp8, x32-scaled
        KT8r = kt_pool.tile([P, ND * Sq], fp8, name="ktr")
        V = v_pool.tile([P, NL, D], bf16)
        Vr = v_pool.tile([P, NL * D], bf16, name="vr")
        PsT = pst_pool.tile([P, NS, Sq], bf16)
        maskT = maskt_pool.tile([P, NS, Sq], bf16)
        # two denominator accumulators so the serial add chain splits
        # across DVE (even kt) and gpsimd (odd kt)
        den_a = den_pool.tile([P, Sq], f32, name="den_a")
        den_b = den_pool.tile([P, Sq], f32, name="den_b")

        # ---- input loads (queue placement = need time) ----
        with nc.named_scope("loads"):
            # K-proj gate = max(xT8, Wk8): split Wk8 across both HWDGE
            # queues so the gate lands ~2us earlier
            nc.sync.dma_start(out=xT8[:, :, :], in_=xt8_d[:, :])
            nc.scalar.dma_start(out=Wk8[:, 0:4, :], in_=wk8_d[:, 0 : 4 * D])
            nc.sync.dma_start(out=Wk8[:, 4:8, :], in_=wk8_d[:, 4 * D : 8 * D])
            nc.scalar.dma_start(out=Wq8[:, :, :], in_=wq8_d[:, :])
            # gpsimd SWDGE: V-path + mask loads
            nc.gpsimd.dma_start(out=xT16[:, :, :], in_=xt16_d[:, :])
            nc.gpsimd.dma_start(out=Wv[:, :, :], in_=wv16_d[:, :])
            nc.gpsimd.dma_start(out=maskT[:, :, :], in_=maskt_d[:, :])

        # ---- K-proj (fp8 DoubleRow): KT8[p,dt,s] = 32*(K[s, dt*128+p]+bk),
        #      local rows only, evicted straight to fp8. Each dt-half is
        #      staged for the AG as soon as its last eviction lands. ----
        def proj_fp8(dst, W8, bias_col0, span, scope, piece_hook=None):
            with nc.named_scope(scope):
                for dt in range(ND):
                    ps = mm_psum.tile([P, span], f32, tag="mm")
                    for ep in range(0, NE, 2):
                        for c0, cw in _chunks(span, NCH):
                            nc.tensor.matmul(
                                ps[:, c0 : c0 + cw],
                                W8[:, ep : ep + 2, dt * P : (dt + 1) * P],
                                xT8[:, ep : ep + 2, c0 : c0 + cw],
                                start=(ep == 0),
                                stop=(ep == NE - 2),
                                perf_mode=DR,
                            )
                    nc.scalar.activation(
                        dst[:, dt, 0:span],
                        ps[:, 0:span],
                        AF.Identity,
                        bias=bqk_t[:, bias_col0 + dt : bias_col0 + dt + 1],
                    )
                    if piece_hook is not None:
                        piece_hook(dt)

        proj_fp8(KT8, Wk8, ND, Sq, "KT")
        if use_cc:
            # stage the whole local K on sync (sync carries ONLY the K
            # exchange until the V readbacks, so nothing K-AG-gated can
            # delay an unrelated transfer via queue-FIFO ordering)
            with nc.named_scope("kx"):
                nc.sync.dma_start(
                    out=ccK_in[:, :].rearrange("p (dt s) -> p dt s", dt=ND),
                    in_=KT8[:, :, :],
                )
                nc.gpsimd.collective_compute(
                    "AllGather",
                    ALU.bypass,
                    replica_groups=GROUPS,
                    ins=[ccK_in[:, :].opt()],
                    outs=[ccK_out[:, :, :].opt()],
                )

        # ---- V-proj (bf16): V[p, st, d], local st=0..7; stage each tile as
        #      soon as it's evicted, single AG for all 8 ----
        with nc.named_scope("V"):
            for st in range(NL):
                ps = mm_psum.tile([P, D], f32, tag="mm")
                for e in range(NE):
                    for c0, cw in _chunks(D, NCH):
                        nc.tensor.matmul(
                            ps[:, c0 : c0 + cw],
                            xT16[:, e, st * P : (st + 1) * P],
                            Wv[:, e, c0 : c0 + cw],
                            start=(e == 0),
                            stop=(e == NE - 1),
                        )
                nc.vector.tensor_tensor(
                    V[:, st, :], ps[:, 0:D], bv_t[:, :], op=ALU.add
                )
                # stage in 2 big HWDGE chunks (not 8 small SWDGE ones), on
                # SCALAR: the V AG trigger must fire BEFORE the K AG
                # completes or the V AG pays ~12us of idle-ncfw latency.
                # (Not sync: Tile may schedule the K-AG-gated readbacks
                # ahead of a later-emitted staging on the same queue.)
                if use_cc and st in (NL // 2 - 1, NL - 1):
                    c = 0 if st < NL // 2 else NL // 2
                    nc.scalar.dma_start(
                        out=ccV_in[:, c * D : (c + NL // 2) * D].rearrange(
                            "p (st d) -> p st d", st=NL // 2
                        ),
                        in_=V[:, c : c + NL // 2, :],
                    )
        if use_cc:
            with nc.named_scope("vx"):
                nc.gpsimd.collective_compute(
                    "AllGather", ALU.bypass, replica_groups=GROUPS,
                    ins=[ccV_in[:, :].opt()], outs=[ccV_out[:, :, :].opt()],
                )

        # ---- K partner readback: the key axis is LOCAL-RELATIVE (own
        #      rows = kt 0..7, partner rows = kt 8..15; the host rotates
        #      the mask to match). Own K is already in KT8[:, :, 0:Sq]
        #      from K-proj, so only the partner slot is fetched — one
        #      per-partition indirect gather (row = roff[p] = (1-h)*128+p)
        #      instead of a 2MB both-slots readback that contends with
        #      the V AllGather for SDMA bandwidth. Scores kt 0..7 need no
        #      readback at all, so scores starts right at Q-proj end. ----
        with nc.named_scope("kin") if use_cc else _nullcm():
            if use_cc:
                import concourse.bass as bass

                nc.gpsimd.indirect_dma_start(
                    out=KT8r[:, :],
                    out_offset=None,
                    in_=ccK_out[:, :, :].rearrange("w p f -> (w p) f"),
                    in_offset=bass.IndirectOffsetOnAxis(
                        ap=roff_t[:, 0:1], axis=0
                    ),
                )

        # ---- Q-proj (fp8 DoubleRow) while the K gather flies ----
        proj_fp8(QT8, Wq8, 0, Sq, "QT")

        # ---- scores (fp8 DoubleRow, transposed) + exp + mask; den
        #      accumulation on gpsimd so DVE only does the mask-mult ----
        KT8r_v = KT8r[:, :].rearrange("p (dt s) -> p dt s", dt=ND)
        with nc.named_scope("scores"):
            for kt in range(NS):
                kT = (
                    KT8 if kt < NL else KT8r_v
                )
                kb = kt if kt < NL else kt - NL
                ps = mm_psum.tile([P, Sq], f32, tag="mm")
                for dp in range(0, ND, 2):
                    for c0, cw in _chunks(Sq, NCH):
                        nc.tensor.matmul(
                            ps[:, c0 : c0 + cw],
                            kT[:, dp : dp + 2, kb * P : (kb + 1) * P],
                            QT8[:, dp : dp + 2, c0 : c0 + cw],
                            start=(dp == 0),
                            stop=(dp == ND - 2),
                            perf_mode=DR,
                        )
                ex = evict.tile([P, Sq], bf16, tag="exp")
                nc.scalar.activation(ex[:, :], ps[:, 0:Sq], AF.Exp, scale=EXP_SCALE)
                nc.vector.tensor_tensor(
                    PsT[:, kt, :], ex[:, :], maskT[:, kt, :], op=ALU.mult
                )
                # gpsimd gets the EVEN kts (its slow initial cast lands
                # early; its last add is kt14, off the critical tail);
                # DVE's odd chain ends at kt15 just ~0.6us after mult15
                eng, den = (nc.gpsimd, den_a) if kt % 2 == 0 else (nc.vector, den_b)
                if kt < 2:
                    eng.tensor_copy(den[:, :], PsT[:, kt, :])
                else:
                    eng.tensor_tensor(
                        den[:, :], den[:, :], PsT[:, kt, :], op=ALU.add
                    )

        # ---- denominators (no V dependency) ----
        rdens = []
        with nc.named_scope("den"):
            for qt in range(NQ):
                dps = den_psum.tile([P, 1], f32, tag="den")
                nc.tensor.matmul(
                    dps[:, 0:1],
                    den_a[:, qt * P : (qt + 1) * P],
                    ones_col[:, 0:1],
                    start=True,
                    stop=False,
                )
                nc.tensor.matmul(
                    dps[:, 0:1],
                    den_b[:, qt * P : (qt + 1) * P],
                    ones_col[:, 0:1],
                    start=False,
                    stop=True,
                )
                rden = den_pool.tile([P, 1], f32, tag=f"rden{qt}", bufs=1)
                nc.vector.reciprocal(rden[:, 0:1], dps[:, 0:1])
                rdens.append(rden)

        # ---- V gather readbacks: emitted AFTER scores so their engine-
        #      queue slots can't block exp/mult issue. PV consumption
        #      order: slot0 whole on sync (idle right after the kin
        #      readbacks), slot1 halves on gpsimd (free after its den
        #      adds) and scalar (free after the last exp). ----
        with nc.named_scope("vin") if use_cc else _nullcm():
            if use_cc:
                # partner V only (own V is already in V[:, 0:8] from
                # V-proj): one per-partition indirect gather, emitted
                # after scores so it can't block the den adds
                import concourse.bass as bass

                nc.gpsimd.indirect_dma_start(
                    out=Vr[:, :],
                    out_offset=None,
                    in_=ccV_out[:, :, :].rearrange("w p f -> (w p) f"),
                    in_offset=bass.IndirectOffsetOnAxis(
                        ap=roff_t[:, 0:1], axis=0
                    ),
                )

        # ---- P@V per query tile (bf16) ----
        Vr_v = Vr[:, :].rearrange("p (st d) -> p st d", st=NL)
        with nc.named_scope("pv"):
            for qt in range(NQ):
                ops = mm_psum.tile([P, D], f32, tag="mm")
                for kt in range(NS):
                    vT = V if kt < NL else Vr_v
                    vb = kt if kt < NL else kt - NL
                    for c0, cw in _chunks(D, NCH):
                        nc.tensor.matmul(
                            ops[:, c0 : c0 + cw],
                            PsT[:, kt, qt * P : (qt + 1) * P],
                            vT[:, vb, c0 : c0 + cw],
                            start=(kt == 0),
                            stop=(kt == NS - 1),
                        )
                ot = o_pool.tile([P, D], f32, tag="o")
                nc.scalar.activation(
                    ot[:, :], ops[:, 0:D], AF.Copy, scale=rdens[qt][:, 0:1]
                )
                if qt < NQ - 1:
                    eng = nc.sync if qt % 2 == 0 else nc.scalar
                    eng.dma_start(out=out_d[qt * P : (qt + 1) * P, :], in_=ot[:, :])
                else:
                    nc.sync.dma_start(
                        out=out_d[qt * P : qt * P + 64, :], in_=ot[0:64, :]
                    )
                    nc.scalar.dma_start(
                        out=out_d[qt * P + 64 : (qt + 1) * P, :], in_=ot[64:P, :]
                    )

    nc.compile()
    return nc


_NC_CACHE = {}


def _get_nc(key=(2048, 1024, 1024, 1024)):
    if key not in _NC_CACHE:
        _NC_CACHE[key] = build_nc(*key)
    return _NC_CACHE[key]


def shard_inputs(x, mask, Wq, bq, Wk, bk, Wv, bv):
    """Host-side prep: pre-transpose/pre-cast per-core inputs.

    The key axis on every core is the GLOBAL batch order (the AllGather
    recomposes K/V in rank order), so the mask is never rotated; each core
    takes its own query rows only.
    """
    import ml_dtypes

    fp8 = ml_dtypes.float8_e4m3
    bf16 = ml_dtypes.bfloat16
    Sq = x.shape[1] // 2
    ND = QD // P

    def pmajor(a):
        # [chunks*128, inner] -> [128, chunks*inner] partition-major
        n, inner = a.shape[0] // P, a.shape[1]
        return np.ascontiguousarray(
            a.reshape(n, P, inner).transpose(1, 0, 2).reshape(P, n * inner)
        )

    w8 = {
        "wq8": pmajor((Wq * WSCALE).astype(fp8)),
        "wk8": pmajor((Wk * WSCALE).astype(fp8)),
        "wv16": pmajor(Wv.astype(bf16)),
    }
    bqk32 = np.ascontiguousarray(
        np.concatenate(
            [(bq * WSCALE).reshape(ND, P).T, (bk * WSCALE).reshape(ND, P).T],
            axis=1,
        ).astype(np.float32)
    )
    bv16 = np.ascontiguousarray(
        np.broadcast_to(bv.reshape(1, -1), (P, bv.size)).astype(bf16)
    )

    in_maps = []
    for c in range(N_CORES):
        b, h = c // 2, c % 2
        xt = x[b, h * Sq : (h + 1) * Sq, :].T  # [E, Sq]
        # key axis LOCAL-RELATIVE per core: own rows first, partner's
        # second (matches KT8/V layout: own from proj, partner gathered)
        mt = mask[b, h * Sq : (h + 1) * Sq, :].T.astype(bf16)  # [S, Sq]
        maskt = np.concatenate(
            [mt[h * Sq : (h + 1) * Sq], mt[(1 - h) * Sq : (2 - h) * Sq]]
        )
        roff = ((1 - h) * P + np.arange(P, dtype=np.int32)).reshape(P, 1)
        in_maps.append(
            {
                "xt16": pmajor(xt.astype(bf16)),
                "xt8": pmajor(xt.astype(fp8)),
                "maskt": pmajor(maskt),
                "bqk32": bqk32,
                "bv16": bv16,
                "roff": np.ascontiguousarray(roff),
                **w8,
            }
        )
    return in_maps


def kernel(**inputs):
    """Full-problem entry point: full unsharded inputs -> full output."""
    from concourse.bass_utils import run_bass_kernel_spmd

    x = np.asarray(inputs["x"], dtype=np.float32)
    mask = np.asarray(inputs["mask"], dtype=np.int32)
    args = [
        np.asarray(inputs[k], dtype=np.float32)
        for k in ("Wq", "bq", "Wk", "bk", "Wv", "bv")
    ]

    nc = _get_nc()
    in_maps = shard_inputs(x, mask, *args)
    res = run_bass_kernel_spmd(nc, in_maps, core_ids=list(range(N_CORES)))

    Sq = S_FULL // 2
    out = np.empty((B, S_FULL, QD), dtype=np.float32)
    for c, r in enumerate(res.results):
        b, h = c // 2, c % 2
        out[b, h * Sq : (h + 1) * Sq, :] = r["out"]
    return out
